# revision 1
# baseline (speedup 1.0000x reference)
"""DHPF kernel for Trainium2: batch-parallel 2D FFT high-pass filter.

Composed-operator formulation: since keep = 1 - in_r (x) in_c,
  y = x - ifft2(box . F) = x - S @ x @ S^T,   S = INV . diag(in_r) . FWD,
where S is ONE dense complex 512x512 matrix, built on-device once per
core after the data-dependent cutoff search (S^T = FWDfull^T . diag .
INVfull as 64 accumulated matmuls).  Each channel is then just
  xT -> U = S@xT -> U^T -> A = S@U^T (natural layout) -> |x - A|,
96 matmuls + 48 transposes per channel with no radix partials or
butterfly combines.  The cutoff search itself still runs a radix-4x128
FFT of channel 8 (box-energy sums as selection-matrix matmuls).
"""

import numpy as np
from contextlib import ExitStack

import jax
import concourse.bass as bass
import concourse.bacc as bacc
import concourse.mybir as mybir
from concourse.tile import TileContext

P = 128
NT = 4  # 512 / 128
H = W = 512
C = 16
B = 8
NCORES = 8
ENERGY = 0.9

F32 = mybir.dt.float32
F32R = mybir.dt.float32r
ALU = mybir.AluOpType
ACTF = mybir.ActivationFunctionType
AX = mybir.AxisListType


# ----------------------------------------------------------------- host consts
def _host_constants():
    consts = {}

    ident = np.eye(P, dtype=np.float32)
    consts["ident"] = ident

    # CT(4,128) stage matrices for the cutoff-search FFT.
    n2 = np.arange(P, dtype=np.float64)
    k2 = np.arange(P, dtype=np.float64)
    fwd_r, fwd_i = [], []
    for k1 in range(NT):
        phF = -2.0 * np.pi * np.outer(n2, k1 + 4.0 * k2) / H
        Mf = np.exp(1j * phF)
        fwd_r.append(Mf.real)
        fwd_i.append(Mf.imag)
    consts["FWDr"] = np.concatenate(fwd_r, axis=1).astype(np.float32)
    consts["FWDi"] = np.concatenate(fwd_i, axis=1).astype(np.float32)
    consts["FWDrn"] = -consts["FWDr"]
    consts["FWDin"] = -consts["FWDi"]

    # Full 512x512 DFT/IDFT matrices in natural (t p) row layout,
    # arr[p, t*512 + n] = M[t*128 + p, n], for the on-device S build.
    kk = np.arange(H, dtype=np.float64)
    ph = 2.0 * np.pi * np.outer(kk, kk) / H

    def tp_layout(M):
        return np.ascontiguousarray(
            M.reshape(NT, P, H).transpose(1, 0, 2).reshape(P, NT * H)
        ).astype(np.float32)

    consts["FWDFr"] = tp_layout(np.cos(ph))
    consts["FWDFi"] = tp_layout(-np.sin(ph))
    consts["FWDFin"] = tp_layout(np.sin(ph))
    consts["INVFr"] = tp_layout(np.cos(ph) / H)
    consts["INVFi"] = tp_layout(np.sin(ph) / H)

    freqmap = (np.arange(H) // P) + 4 * (np.arange(H) % P)  # stored idx -> freq

    # Asel[r, cidx] = 1 if row r in R(c=cidx+1) = [0,c) u [512-c,512); col 255 = all ones
    Asel = np.zeros((H, 256), dtype=np.float32)
    for cidx in range(255):
        c = cidx + 1
        Asel[:c, cidx] = 1.0
        Asel[H - c :, cidx] = 1.0
    Asel[:, 255] = 1.0
    consts["Asel"] = Asel[freqmap]

    Bsel = np.zeros((256, W), dtype=np.float32)
    for cidx in range(255):
        c = cidx + 1
        Bsel[cidx, :c] = 1.0
        Bsel[cidx, W - c :] = 1.0
    Bsel[255, :] = 1.0
    consts["Bsel"] = Bsel[:, freqmap]

    # natural-layout frequency iota: value t*128+p at [p, t]
    iota_n = np.zeros((P, NT), dtype=np.float32)
    for t in range(NT):
        iota_n[:, t] = t * P + np.arange(P)
    consts["iota_n"] = iota_n

    consts["ones_col"] = np.ones((P, 1), dtype=np.float32)
    consts["ones_row"] = np.ones((1, P), dtype=np.float32)
    bc127 = np.zeros((P, P), dtype=np.float32)
    bc127[127, :] = ENERGY
    consts["bc127"] = bc127
    return consts


CONST_DTYPES = {
    "FWDr": F32R, "FWDi": F32R, "FWDrn": F32R, "FWDin": F32R,
    "FWDFr": F32R, "FWDFi": F32R, "FWDFin": F32R, "INVFr": F32R, "INVFi": F32R,
    "ident": F32R, "Asel": F32R, "bc127": F32,
    "Bsel": F32, "iota_n": F32, "ones_col": F32, "ones_row": F32,
}

SHAPES = {
    "FWDr": [P, W], "FWDi": [P, W], "FWDrn": [P, W], "FWDin": [P, W],
    "FWDFr": [P, NT * W], "FWDFi": [P, NT * W], "FWDFin": [P, NT * W],
    "INVFr": [P, NT * W], "INVFi": [P, NT * W],
    "ident": [P, P], "Asel": [H, 256], "Bsel": [256, W],
    "iota_n": [P, NT], "ones_col": [P, 1], "ones_row": [1, P], "bc127": [P, P],
}


def _blk(mat, t, blk=512):
    return mat[:, t * blk : (t + 1) * blk]


def build_nc():
    nc = bacc.Bacc()
    x_d = nc.declare_dram_parameter("x", [C, H, W], F32, isOutput=False)
    cd = {}
    for name, shp in SHAPES.items():
        cd[name] = nc.declare_dram_parameter(name, shp, CONST_DTYPES[name], isOutput=False)
    out_d = nc.declare_dram_parameter("out", [C, H, W], F32, isOutput=True)

    with ExitStack() as ctx:
        tc = ctx.enter_context(TileContext(nc))
        cpool = ctx.enter_context(tc.tile_pool(name="consts", bufs=1))
        persist = ctx.enter_context(tc.tile_pool(name="persist", bufs=1))
        work = ctx.enter_context(tc.tile_pool(name="work", bufs=1))
        mpool = ctx.enter_context(tc.tile_pool(name="mp", bufs=2))
        ppool = ctx.enter_context(tc.tile_pool(name="pp", bufs=1))
        fpool = ctx.enter_context(tc.tile_pool(name="fp", bufs=1))
        xpool = ctx.enter_context(tc.tile_pool(name="xp", bufs=3))
        psmm = ctx.enter_context(tc.tile_pool(name="psmm", bufs=4, space="PSUM"))
        pstp = ctx.enter_context(tc.tile_pool(name="pstp", bufs=3, space="PSUM"))
        pssm = ctx.enter_context(tc.tile_pool(name="pssm", bufs=1, space="PSUM"))

        cs = {}

        def load_const(name):
            t = cpool.tile(SHAPES[name], CONST_DTYPES[name], tag=name)
            nc.sync.dma_start(t[:], cd[name].ap())
            cs[name] = t

        def load_x(ch):
            xt = xpool.tile([P, NT * W], F32R, tag="xa")
            nc.gpsimd.dma_start(
                xt[:].rearrange("p (t j) -> p t j", t=NT),
                x_d.ap()[ch].rearrange("(t p) j -> p t j", p=P),
            )
            return xt

        # x8 + the cutoff-FFT constants first; everything else during phase A.
        x8 = load_x(8)
        for name in ("FWDr", "FWDi", "FWDrn", "FWDin", "ident"):
            load_const(name)

        # ---- helpers ------------------------------------------------------
        def evac_copy(dst, src):
            nc.scalar.copy(dst, src)

        def tt_op(dst, a, b, op):
            nc.vector.tensor_tensor(dst, a, b, op)

        def ctM(fam, part, k1):
            return cs[fam + part][:, k1 * P : (k1 + 1) * P]

        def _accmm(dst_blk, plan):
            ps = psmm.tile([P, W], F32, tag="ps")
            n = len(plan)
            for i, (l, r) in enumerate(plan):
                nc.tensor.matmul(ps[:], l, r, start=(i == 0), stop=(i == n - 1))
            evac_copy(dst_blk, ps[:])

        def real_partials(xt):
            sm = fpool.tile([P, 2 * W], F32R, tag="p_s")
            df = fpool.tile([P, 2 * W], F32R, tag="p_d")
            tt_op(sm[:], xt[:, : 2 * W], xt[:, 2 * W :], ALU.add)
            tt_op(df[:], xt[:, : 2 * W], xt[:, 2 * W :], ALU.subtract)
            return {"s02": sm[:, :W], "s13": sm[:, W:], "d02": df[:, :W], "d13": df[:, W:]}

        def ct_fwd_real(parts, tags):
            ar = mpool.tile([P, NT * W], F32R, tag=tags[0])
            ai = mpool.tile([P, NT * W], F32R, tag=tags[1])
            s02, d02, s13, d13 = parts["s02"], parts["d02"], parts["s13"], parts["d13"]
            plans = {
                0: (([("r", s02), ("r", s13)]), ([("i", s02), ("i", s13)])),
                1: (([("r", d02), ("i", d13)]), ([("i", d02), ("rn", d13)])),
                2: (([("r", s02), ("rn", s13)]), ([("i", s02), ("in", s13)])),
                3: (([("r", d02), ("in", d13)]), ([("i", d02), ("r", d13)])),
            }
            for k1 in range(NT):
                pre, pim = plans[k1]
                _accmm(_blk(ar, k1), [(ctM("FWD", v, k1), op) for v, op in pre])
                _accmm(_blk(ai, k1), [(ctM("FWD", v, k1), op) for v, op in pim])
            return ar, ai

        def transpose_to_partials(srcr, srci):
            out = {}
            for plane, src in (("r", srcr), ("i", srci)):
                sb = {}
                for jt in range(NT):
                    pst = pstp.tile([P, W], F32R, tag="tp")
                    for it in range(NT):
                        nc.tensor.transpose(
                            pst[:, it * P : (it + 1) * P],
                            src[:, it * W + jt * P : it * W + jt * P + P],
                            cs["ident"][:],
                        )
                    if jt < 2:
                        t = work.tile([P, W], F32R, tag=f"tb{jt}{plane}")
                        evac_copy(t[:], pst[:])
                        sb[jt] = t
                    else:
                        base = sb[jt - 2]
                        pa = ppool.tile([P, W], F32R, tag=f"p{jt - 2}{jt}{plane}+")
                        pb = ppool.tile([P, W], F32R, tag=f"p{jt - 2}{jt}{plane}-")
                        tt_op(pa[:], base[:], pst[:], ALU.add)
                        tt_op(pb[:], base[:], pst[:], ALU.subtract)
                        out[f"p{jt - 2}{jt}{plane}"] = pa
                        out[f"d{jt - 2}{jt}{plane}"] = pb
            return out

        def ct_fwd_from_partials(parts, tags):
            orr = mpool.tile([P, NT * W], F32R, tag=tags[0])
            oii = mpool.tile([P, NT * W], F32R, tag=tags[1])
            p02r, p02i = parts["p02r"], parts["p02i"]
            d02r, d02i = parts["d02r"], parts["d02i"]
            p13r, p13i = parts["p13r"], parts["p13i"]
            d13r, d13i = parts["d13r"], parts["d13i"]
            plans = {
                0: ([("r", p02r), ("r", p13r), ("in", p02i), ("in", p13i)],
                    [("i", p02r), ("i", p13r), ("r", p02i), ("r", p13i)]),
                2: ([("r", p02r), ("rn", p13r), ("in", p02i), ("i", p13i)],
                    [("i", p02r), ("in", p13r), ("r", p02i), ("rn", p13i)]),
                1: ([("r", d02r), ("r", d13i), ("in", d02i), ("i", d13r)],
                    [("i", d02r), ("i", d13i), ("r", d02i), ("rn", d13r)]),
                3: ([("r", d02r), ("rn", d13i), ("in", d02i), ("in", d13r)],
                    [("i", d02r), ("in", d13i), ("r", d02i), ("r", d13r)]),
            }
            for k1 in range(NT):
                pre, pim = plans[k1]
                _accmm(_blk(orr, k1), [(ctM("FWD", v, k1), op[:]) for v, op in pre])
                _accmm(_blk(oii, k1), [(ctM("FWD", v, k1), op[:]) for v, op in pim])
            return orr, oii

        def transpose_mat(src, tag, pool=None):
            # evacs alternate DVE/ACT: a single engine's ~600 ns evac gates
            # the 320 ns transpose groups otherwise
            dst = (pool or work).tile([P, NT * W], F32R, tag=tag)
            for jt in range(NT):
                ps = pstp.tile([P, W], F32R, tag="tp")
                for it in range(NT):
                    nc.tensor.transpose(
                        ps[:, it * P : (it + 1) * P],
                        src[:, it * W + jt * P : it * W + jt * P + P],
                        cs["ident"][:],
                    )
                if jt % 2 == 0:
                    nc.vector.tensor_copy(_blk(dst, jt), ps[:])
                else:
                    nc.scalar.copy(_blk(dst, jt), ps[:])
            return dst

        # ---- phase A: channel-8 spectrum + |.|^2 --------------------------
        a8r, a8i = ct_fwd_real(real_partials(x8), ("m1r", "m1i"))
        parts8 = transpose_to_partials(a8r, a8i)
        b8r, b8i = ct_fwd_from_partials(parts8, ("m1r", "m1i"))

        mag = work.tile([P, NT * W], F32R, tag="mag")
        for t in range(NT):
            nc.scalar.square(_blk(b8r, t), _blk(b8r, t))
            nc.scalar.square(_blk(b8i, t), _blk(b8i, t))
            nc.vector.tensor_tensor(_blk(mag, t), _blk(b8r, t), _blk(b8i, t), ALU.add)

        # remaining consts; S-build operands land in reusable channel tiles
        t = cpool.tile([P, NT * 256], F32R, tag="Asel")
        nc.sync.dma_start(
            t[:].rearrange("p (t j) -> p t j", t=NT),
            cd["Asel"].ap().rearrange("(t p) j -> p t j", p=P),
        )
        cs["Asel"] = t
        t = cpool.tile([P, 2 * W], F32, tag="Bsel")
        nc.sync.dma_start(
            t[:].rearrange("p (t j) -> p t j", t=2),
            cd["Bsel"].ap().rearrange("(t p) j -> p t j", p=P),
        )
        cs["Bsel"] = t
        for name in ("bc127", "iota_n", "ones_col", "ones_row"):
            load_const(name)
        fwdf_r = mpool.tile([P, NT * W], F32R, tag="m1r")
        nc.sync.dma_start(fwdf_r[:], cd["FWDFr"].ap())
        fwdf_i = mpool.tile([P, NT * W], F32R, tag="m1i")
        nc.sync.dma_start(fwdf_i[:], cd["FWDFi"].ap())
        fwdf_in = work.tile([P, NT * W], F32R, tag="yi")
        nc.sync.dma_start(fwdf_in[:], cd["FWDFin"].ap())
        invm_r = work.tile([P, NT * W], F32R, tag="w1r")
        nc.sync.dma_start(invm_r[:], cd["INVFr"].ap())
        invm_i = work.tile([P, NT * W], F32R, tag="w1i")
        nc.sync.dma_start(invm_i[:], cd["INVFi"].ap())

        # front(0) transpose: PE filler under the cutoff's dependency chain
        x0 = load_x(0)
        xT0 = transpose_mat(x0, "xt")

        # ---- cutoff search ------------------------------------------------
        e_tiles = []
        for mt in range(2):
            ps = psmm.tile([P, W], F32, tag="ps")
            for kt in range(NT):
                nc.tensor.matmul(
                    ps[:], cs["Asel"][:, kt * 256 + mt * P : kt * 256 + mt * P + P],
                    _blk(mag, kt), start=(kt == 0), stop=(kt == NT - 1),
                )
            msk = work.tile([P, W], F32, tag="msk")
            nc.vector.tensor_tensor(msk[:], ps[:], _blk(cs["Bsel"], mt), ALU.mult)
            ev = persist.tile([P, 1], F32, tag=f"e{mt}")
            nc.vector.tensor_reduce(ev[:], msk[:], op=ALU.add, axis=AX.X)
            e_tiles.append(ev)

        psb = pssm.tile([P, 1], F32, tag="sm")
        nc.tensor.matmul(psb[:], cs["bc127"][:], e_tiles[1][:], start=True, stop=True)
        thr_bc = persist.tile([P, 1], F32, tag="thr_bc")
        nc.any.tensor_copy(thr_bc[:], psb[:])

        nok0 = persist.tile([P, 1], F32, tag="nok0")
        nok1 = persist.tile([P, 1], F32, tag="nok1")
        nc.vector.tensor_scalar(nok0[:], e_tiles[0][:], thr_bc[:], None, ALU.is_lt)
        nc.vector.tensor_scalar(nok1[:], e_tiles[1][:], thr_bc[:], None, ALU.is_lt)
        pcnt = pssm.tile([1, 1], F32, tag="sm")
        nc.tensor.matmul(pcnt[:], nok0[:], cs["ones_col"][:], start=True, stop=False)
        nc.tensor.matmul(pcnt[:], nok1[:127], cs["ones_col"][:127], start=False, stop=True)
        cnt = persist.tile([1, 1], F32, tag="cnt")
        nc.any.tensor_copy(cnt[:], pcnt[:])

        # cval = cnt+1 if cnt < 255 else 5
        aa = persist.tile([1, 1], F32, tag="aa")
        fb = persist.tile([1, 1], F32, tag="fb")
        uu = persist.tile([1, 1], F32, tag="uu")
        cval = persist.tile([1, 1], F32, tag="cval")
        nc.vector.tensor_scalar(aa[:], cnt[:], 1.0, None, ALU.add)
        nc.vector.tensor_scalar(fb[:], cnt[:], 254.5, None, ALU.is_ge)
        nc.vector.tensor_scalar(uu[:], aa[:], 5.0, None, ALU.subtract)
        nc.vector.tensor_tensor(uu[:], uu[:], fb[:], ALU.mult)
        nc.vector.tensor_tensor(cval[:], aa[:], uu[:], ALU.subtract)

        psb2 = pssm.tile([P, 1], F32, tag="sm")
        nc.tensor.matmul(psb2[:], cs["ones_row"][:], cval[:], start=True, stop=True)
        c_bc = persist.tile([P, 1], F32, tag="c_bc")
        nc.any.tensor_copy(c_bc[:], psb2[:])
        c2_bc = persist.tile([P, 1], F32, tag="c2_bc")
        nc.vector.tensor_scalar(c2_bc[:], c_bc[:], -1.0, 512.0, ALU.mult, ALU.add)

        # in_r over natural frequency layout [p, t] (freq = t*128+p)
        in_r = persist.tile([P, NT], F32, tag="in_r")
        tmpr = persist.tile([P, NT], F32, tag="tmpr")
        nc.vector.tensor_scalar(in_r[:], cs["iota_n"][:], c_bc[:], None, ALU.is_lt)
        nc.vector.tensor_scalar(tmpr[:], cs["iota_n"][:], c2_bc[:], None, ALU.is_ge)
        nc.vector.tensor_tensor(in_r[:], in_r[:], tmpr[:], ALU.max)

        # ---- build S^T = FWDfull^T . diag(in_r) . INVfull -----------------
        # scale INVfull rows (freq axis = partitions+blocks) by in_r, in place
        for t in range(NT):
            nc.vector.tensor_scalar(
                _blk(invm_r, t), _blk(invm_r, t), in_r[:, t : t + 1], None, ALU.mult
            )
            nc.vector.tensor_scalar(
                _blk(invm_i, t), _blk(invm_i, t), in_r[:, t : t + 1], None, ALU.mult
            )
        st_r = persist.tile([P, NT * W], F32R, tag="st_r")
        st_i = persist.tile([P, NT * W], F32R, tag="st_i")
        st_in = persist.tile([P, NT * W], F32R, tag="st_in")
        for nb in range(NT):
            ps = psmm.tile([P, W], F32, tag="ps")
            for kb in range(NT):
                nc.tensor.matmul(
                    ps[:], fwdf_r[:, kb * W + nb * P : kb * W + nb * P + P],
                    _blk(invm_r, kb), start=(kb == 0), stop=False,
                )
            for kb in range(NT):
                nc.tensor.matmul(
                    ps[:], fwdf_in[:, kb * W + nb * P : kb * W + nb * P + P],
                    _blk(invm_i, kb), start=False, stop=(kb == NT - 1),
                )
            evac_copy(_blk(st_r, nb), ps[:])
            ps = psmm.tile([P, W], F32, tag="ps")
            for kb in range(NT):
                nc.tensor.matmul(
                    ps[:], fwdf_r[:, kb * W + nb * P : kb * W + nb * P + P],
                    _blk(invm_i, kb), start=(kb == 0), stop=False,
                )
            for kb in range(NT):
                nc.tensor.matmul(
                    ps[:], fwdf_i[:, kb * W + nb * P : kb * W + nb * P + P],
                    _blk(invm_r, kb), start=False, stop=(kb == NT - 1),
                )
            evac_copy(_blk(st_i, nb), ps[:])
        nc.scalar.mul(st_in[:], st_i[:], -1.0)
        st_s = persist.tile([P, NT * W], F32R, tag="st_s")
        nc.vector.tensor_tensor(st_s[:], st_r[:], st_i[:], ALU.add)

        def s_chunk(plane, jb, ob):
            return plane[:, jb * W + ob * P : jb * W + ob * P + P]

        # ---- per-channel pipeline -----------------------------------------
        def u_stage(xT, tags):
            """U = S @ xT (complex from real): 32 matmuls."""
            ur = mpool.tile([P, NT * W], F32R, tag=tags[0])
            ui = mpool.tile([P, NT * W], F32R, tag=tags[1])
            for dst, splane in ((ur, st_r), (ui, st_i)):
                for ob in range(NT):
                    ps = psmm.tile([P, W], F32, tag="ps")
                    for jb in range(NT):
                        nc.tensor.matmul(
                            ps[:], s_chunk(splane, jb, ob), _blk(xT, jb),
                            start=(jb == 0), stop=(jb == NT - 1),
                        )
                    evac_copy(_blk(dst, ob), ps[:])
            return ur, ui

        def a_stage(ch, wr, wi, ws, xt_nat):
            """A = S @ W via 3-mult complex product; y = |x - A| fused."""
            yr = work.tile([P, NT * W], F32, tag="yr")
            yi = work.tile([P, NT * W], F32, tag="yi")
            for ob in range(NT):
                ps1 = psmm.tile([P, W], F32, tag="ps")
                for jb in range(NT):
                    nc.tensor.matmul(
                        ps1[:], s_chunk(st_r, jb, ob), _blk(wr, jb),
                        start=(jb == 0), stop=(jb == NT - 1),
                    )
                t1 = work.tile([P, W], F32, tag="t1")
                evac_copy(t1[:], ps1[:])
                ps2 = psmm.tile([P, W], F32, tag="ps")
                for jb in range(NT):
                    nc.tensor.matmul(
                        ps2[:], s_chunk(st_i, jb, ob), _blk(wi, jb),
                        start=(jb == 0), stop=(jb == NT - 1),
                    )
                ps3 = psmm.tile([P, W], F32, tag="ps")
                for jb in range(NT):
                    nc.tensor.matmul(
                        ps3[:], s_chunk(st_s, jb, ob), _blk(ws, jb),
                        start=(jb == 0), stop=(jb == NT - 1),
                    )
                e1 = work.tile([P, W], F32, tag="e1")
                tt_op(e1[:], _blk(xt_nat, ob), t1[:], ALU.subtract)
                tt_op(_blk(yr, ob), e1[:], ps2[:], ALU.add)
                e2 = work.tile([P, W], F32, tag="e2")
                tt_op(e2[:], ps3[:], t1[:], ALU.subtract)
                tt_op(_blk(yi, ob), e2[:], ps2[:], ALU.subtract)
            # |y| = sqrt((x-A_r)^2 + A_i^2)
            if ch < C - 1:
                nc.scalar.square(yr[:], yr[:])
                nc.scalar.square(yi[:], yi[:])
                nc.vector.tensor_tensor(yr[:], yr[:], yi[:], ALU.add)
                nc.scalar.sqrt(yr[:], yr[:])
                nc.sync.dma_start(
                    out_d.ap()[ch].rearrange("(t p) j -> p t j", p=P),
                    yr[:].rearrange("p (t j) -> p t j", t=NT),
                )
            else:
                for t in range(NT):
                    nc.scalar.square(_blk(yr, t), _blk(yr, t))
                    nc.scalar.square(_blk(yi, t), _blk(yi, t))
                    nc.vector.tensor_tensor(_blk(yr, t), _blk(yr, t), _blk(yi, t), ALU.add)
                    nc.scalar.sqrt(_blk(yr, t), _blk(yr, t))
                    nc.sync.dma_start(
                        out_d.ap()[ch][t * P : (t + 1) * P], _blk(yr, t)
                    )

        u0 = u_stage(xT0, ("m1r", "m1i"))
        xs = {0: x0, 1: load_x(1)}
        us = {0: u0}
        for ch in range(C):
            if ch + 2 < C:
                xs[ch + 2] = load_x(ch + 2)
            if ch + 1 < C:
                xTn = transpose_mat(xs[ch + 1], "xt")
            ur, ui = us.pop(ch)
            wr = transpose_mat(ur, "w1r")
            wi = transpose_mat(ui, "w1i")
            ws = work.tile([P, NT * W], F32R, tag="w1s")
            tt_op(ws[:], wr[:], wi[:], ALU.add)
            if ch + 1 < C:
                us[ch + 1] = u_stage(xTn, ("m1r", "m1i"))
            a_stage(ch, wr, wi, ws, xs.pop(ch))

    nc.compile()
    return nc


# ----------------------------------------------------------------- pjrt runner
_CACHE = {}


def _make_runner():
    """Compile once; returns callable taking full x [8,16,512,512] -> [8,16,512,512]."""
    from jax.sharding import Mesh, PartitionSpec
    from jax.experimental.shard_map import shard_map
    from concourse.bass2jax import _bass_exec_p, install_neuronx_cc_hook, partition_id_tensor

    install_neuronx_cc_hook()
    nc = build_nc()
    consts = _host_constants()

    partition_name = nc.partition_id_tensor.name if nc.partition_id_tensor else None
    in_names = []
    out_names = []
    out_avals = []
    for alloc in nc.m.functions[0].allocations:
        if not isinstance(alloc, mybir.MemoryLocationSet):
            continue
        name = alloc.memorylocations[0].name
        if alloc.kind == "ExternalInput":
            if name != partition_name:
                in_names.append(name)
        elif alloc.kind == "ExternalOutput":
            out_names.append(name)
            out_avals.append(
                jax.core.ShapedArray(tuple(alloc.tensor_shape), mybir.dt.np(alloc.dtype))
            )
    n_params = len(in_names)
    n_outs = len(out_avals)
    all_names = in_names + out_names
    if partition_name is not None:
        all_names = all_names + [partition_name]

    def _body(*args):
        operands = list(args)
        if partition_name is not None:
            operands.append(partition_id_tensor())
        outs = _bass_exec_p.bind(
            *operands,
            out_avals=tuple(out_avals),
            in_names=tuple(all_names),
            out_names=tuple(out_names),
            lowering_input_output_aliases=(),
            sim_require_finite=True,
            sim_require_nnan=True,
            nc=nc,
        )
        return tuple(outs)

    devices = jax.devices()[:NCORES]
    mesh = Mesh(np.asarray(devices), ("core",))
    donate = tuple(range(n_params, n_params + n_outs))
    sharded = jax.jit(
        shard_map(
            _body,
            mesh=mesh,
            in_specs=(PartitionSpec("core"),) * (n_params + n_outs),
            out_specs=(PartitionSpec("core"),) * n_outs,
            check_rep=False,
        ),
        donate_argnums=donate,
        keep_unused=True,
    )

    from jax.sharding import NamedSharding
    import jax.numpy as jnp

    shard = NamedSharding(mesh, PartitionSpec("core"))

    consts_dev = {}
    for name in in_names:
        if name == "x":
            continue
        consts_dev[name] = jax.device_put(
            np.concatenate([consts[name]] * NCORES, axis=0), shard
        )

    import os as _os
    import time as _time
    _dbg = _os.environ.get("KERNEL_DEBUG_TIMING")

    def run(x_full):
        t0 = _time.time()
        per_core_inputs = []
        for name in in_names:
            if name == "x":
                xd = jax.device_put(x_full.reshape(NCORES * C, H, W), shard)
                xd.block_until_ready()
                per_core_inputs.append(xd)
            else:
                per_core_inputs.append(consts_dev[name])
        t1 = _time.time()
        zeros = [
            jax.device_put(
                jnp.zeros((NCORES * a.shape[0], *a.shape[1:]), a.dtype), shard
            )
            for a in out_avals
        ]
        for z in zeros:
            z.block_until_ready()
        t2 = _time.time()
        out_arrs = sharded(*per_core_inputs, *zeros)
        for o in out_arrs:
            o.block_until_ready()
        t3 = _time.time()
        globals()["LAST_EXEC_S"] = t3 - t2
        o = np.asarray(out_arrs[out_names.index("out")])
        t4 = _time.time()
        if _dbg:
            print(f"[timing] h2d_x={t1-t0:.3f}s zeros={t2-t1:.3f}s exec={t3-t2:.3f}s d2h={t4-t3:.3f}s")
        return o.reshape(NCORES, C, H, W)

    return run


def kernel(x):
    x = np.ascontiguousarray(np.asarray(x, dtype=np.float32))
    assert x.shape == (B, C, H, W)
    if "run" not in _CACHE:
        _CACHE["run"] = _make_runner()
    return _CACHE["run"](x).astype(np.float32)


if __name__ == "__main__":
    rng = np.random.default_rng(0)
    x = rng.standard_normal((B, C, H, W), dtype=np.float32)
    y = kernel(x)
    print(y.shape, y.dtype, float(y.mean()))

